# revision 1
# baseline (speedup 1.0000x reference)
"""Trainium2 Bass kernel for nn_MOA_13254269075617 (sparse windowed attention block).

Sharding: data-parallel over batch B=8 across 8 NeuronCores (1 image each).
BatchNorm uses global batch stats via an on-device AllReduce of per-channel
sum / sum-of-squares.

Per-core pipeline (all in the spatially-TRANSPOSED frame T(z)[u,v]=z[v,u],
which makes the reference's final transpose a no-op):
  x_cm   : x in channel-major [256, 4096] (original grid), via PE transposes
  vT_tm  : (xT @ Wv + bv) token-major [4096, 256] (transposed grid), bf16
  A      : softmax(x @ Wa + ba) pq-major [81, 4096], zero-padded 66-pitch grid
  W      : 25-tap position-varying stencil weights, built from A with 9
           shifted selector matmuls (fold+attention combined algebraically)
  xfT    : 25-tap stencil apply, token-major FMAs on DVE + GPSIMD
  x1/x2  : relu chains with 3x3/5x5 maxpools (separable shifted-max trees)
  out    : concat-matmul (Wfu) + residual, BN with AllReduce'd stats
"""
import sys

for _p in (
    "/root/.axon_site",
    "/root/.axon_site/_ro/trn_rl_repo",
    "/root/.axon_site/_ro/pypackages",
    "/opt/trn_rl_repo",
):
    if _p not in sys.path:
        sys.path.append(_p)

from itertools import product

import numpy as np

import concourse.bass as bass
import concourse.tile as tile
from concourse import bacc, mybir
from concourse.bass_utils import run_bass_kernel_spmd

F32 = mybir.dt.float32
F32R = mybir.dt.float32r
BF16 = mybir.dt.bfloat16
ALU = mybir.AluOpType
ACT = mybir.ActivationFunctionType

B, H, W, C = 8, 64, 64, 256
L = H * W                      # 4096 tokens
NCHUNK = L // 128              # 32 token chunks
N_CORES = 8
EPS = 1e-5


def _r(ap):
    return ap.bitcast(F32R)


def host_consts():
    """Selector matrices and small constants (host-precomputed, same all cores)."""
    selsum = np.zeros((81, 9), np.float32)
    for p in range(9):
        selsum[9 * p:9 * p + 9, p] = 1.0
    selrep = np.zeros((9, 81), np.float32)
    for p in range(9):
        selrep[p, 9 * p:9 * p + 9] = 1.0
    # selshift[:, 25*d + tap]: for (di,dj) block d, tap (e,f):
    #   k = 9*(3di+dj) + 3(di+e)+(dj+f) if di+e,dj+f in [0,3)
    selshift = np.zeros((81, 9 * 25), np.float32)
    for d, (di, dj) in enumerate(product(range(3), range(3))):
        for t, (e, f) in enumerate(product(range(-2, 3), range(-2, 3))):
            dip, djp = di + e, dj + f
            if 0 <= dip < 3 and 0 <= djp < 3:
                k = 9 * (3 * di + dj) + (3 * dip + djp)
                selshift[k, 25 * d + t] = 1.0
    DELTAS = (-2, -1, 1, 2, 62, 63, 64, 65, 66, -62, -63, -64, -65, -66)
    shifts = np.zeros((128, 28, 128), np.float32)
    for fi, f in enumerate(DELTAS):
        for m in range(128):
            k = m + f
            if 0 <= k < 128:
                shifts[k, 2 * fi, m] = 1.0          # main block
            elif f > 0:
                shifts[k - 128, 2 * fi + 1, m] = 1.0  # carry from chunk j+1
            else:
                shifts[k + 128, 2 * fi + 1, m] = 1.0  # carry from chunk j-1
    wmask = np.ones((25, 64, 64), np.float32)
    for t, (e, f) in enumerate(product(range(-2, 3), range(-2, 3))):
        if e > 0: wmask[t, 64 - e:, :] = 0
        if e < 0: wmask[t, :-e, :] = 0
        if f > 0: wmask[t, :, 64 - f:] = 0
        if f < 0: wmask[t, :, :-f] = 0
    return {
        "selsum": selsum,
        "selrep": selrep,
        "selshift": selshift,
        "wmask": wmask.reshape(25, 4096),
        "shifts": shifts.reshape(128, 28 * 128),
        "ident": np.eye(128, dtype=np.float32),
        "ones1": np.ones((1, 128), np.float32),
    }


def build(nc, n_cores, debug=False):
    d = {}
    def din(name, shape):
        d[name] = nc.dram_tensor(name, list(shape), F32, kind="ExternalInput").ap()
    def dout(name, shape):
        d[name] = nc.dram_tensor(name, list(shape), F32, kind="ExternalOutput").ap()

    d["xbf"] = nc.dram_tensor("xbf", [L, C], BF16, kind="ExternalInput").ap()
    din("wv", (C, C)); din("bv", (1, C))
    din("wa", (C, 81)); din("ba", (81, 1))
    din("wfu", (2 * C, C)); din("bfu2", (128, 2))
    din("gamma2", (128, 2)); din("beta2", (128, 2))
    din("selsum", (81, 9)); din("selrep", (9, 81)); din("selshift", (81, 225))
    din("ident", (128, 128)); din("ones1", (1, 128)); din("wmask", (25, L))
    din("shifts", (128, 28 * 128))
    dout("y", (L, C))
    if debug:
        dout("dbg_xcm", (2 * 128, L))
        dout("dbg_vt", (L, C))
        dout("dbg_ae", (81, 4356))
        dout("dbg_w", (25, L))
        dout("dbg_xf", (2 * 128, L))
        dout("dbg_x1", (2 * 128, L))
        dout("dbg_out", (2 * 128, L))

    with tile.TileContext(nc) as tc:
        _build_tc(tc, d, n_cores, debug)
    return d


def _build_tc(tc, d, n_cores, debug):
    nc = tc.nc
    from contextlib import ExitStack
    es = ExitStack()
    with es:
        consts = es.enter_context(tc.tile_pool(name="consts", bufs=1))
        main = es.enter_context(tc.tile_pool(name="main", bufs=1))
        dram = es.enter_context(tc.tile_pool(name="dram", bufs=2, space="DRAM"))

        # ---- const loads ----
        def cload(name, shape):
            t = consts.tile(list(shape), F32, tag=name, name=name)
            nc.sync.dma_start(t[:], d[name][:])
            return t
        ident = cload("ident", (128, 128))
        def cload_bf(name, shape):
            t = consts.tile(list(shape), BF16, tag=name, name=name)
            nc.gpsimd.dma_start(t[:], d[name][:])
            return t
        ones1 = cload_bf("ones1", (1, 128))
        bv_sb = cload_bf("bv", (1, C))
        ba_sb = cload("ba", (81, 1))
        selsum = cload_bf("selsum", (81, 9))
        selrep_bf = cload_bf("selrep", (9, 81))
        selshift = cload_bf("selshift", (81, 225))
        bfu2 = cload("bfu2", (128, 2))
        gamma2 = cload("gamma2", (128, 2))
        beta2 = cload("beta2", (128, 2))
        wv_sb = consts.tile([128, 2, C], BF16, tag="wv", name="wv_sb")
        for kc in range(2):
            nc.gpsimd.dma_start(wv_sb[:, kc, :], d["wv"][128 * kc:128 * (kc + 1), :])
        wa_sb = consts.tile([128, 2, 81], BF16, tag="wa", name="wa_sb")
        for kc in range(2):
            nc.gpsimd.dma_start(wa_sb[:, kc, :], d["wa"][128 * kc:128 * (kc + 1), :])
        shifts_sb = consts.tile([128, 28, 128], BF16, tag="shifts", name="shifts_sb")
        nc.gpsimd.dma_start(shifts_sb[:], d["shifts"].rearrange("p (s m) -> p s m", s=28))
        wfu_sb = consts.tile([128, 4, 2, 128], BF16, tag="wfu", name="wfu_sb")
        for kc in range(4):
            for mc in range(2):
                nc.gpsimd.dma_start(
                    wfu_sb[:, kc, mc, :],
                    d["wfu"][128 * kc:128 * (kc + 1), 128 * mc:128 * (mc + 1)])

        # ---- persistent big tensors ----
        # xT_cm: channel-major x in TRANSPOSED-grid token order (l' = u*64+v,
        # column l' holds x[v, u, :]) so every matmul operand is contiguous.
        xT_cm = [main.tile([128, L], BF16, tag=f"x_cm{cc}", name=f"xT_cm{cc}") for cc in range(2)]

        # ---- phase A: load x via transpose-DMAs, then grid permutation ----
        cmAB = tc.tile_pool(name="psAB", bufs=3, space="PSUM"); psAB = cmAB.__enter__()
        xc_tmp = [main.tile([128, L], BF16, tag="s16b", name=f"xc_tmp{cc}",
                            bufs=2) for cc in range(2)]
        for cc in range(2):
            for q in range(8):
                nc.sync.dma_start_transpose(
                    xc_tmp[cc][:, 512 * q:512 * (q + 1)],
                    d["xbf"][512 * q:512 * (q + 1), 128 * cc:128 * (cc + 1)])
        # column permutation l=(h,w) -> l'=(u,v)=(w,h)
        for cc in range(2):
            nc.vector.tensor_copy(
                xT_cm[cc].rearrange("p (u v) -> p u v", u=64),
                xc_tmp[cc].rearrange("p (h w) -> p w h", h=64))
        if debug:
            for cc2 in range(2):
                nc.gpsimd.dma_start(d["dbg_xcm"][128 * cc2:128 * (cc2 + 1), :],
                                    xT_cm[cc2][:])

        # ---- phase B: vT = xT @ Wv + bv, token-major (bf16) ----
        vT = main.tile([128, NCHUNK, C], BF16, tag="vT", name="vT")
        bv256 = consts.tile([128, C], BF16, tag="bv256", name="bv256")
        psb = psAB.tile([128, C], F32, tag="vps", name="vps")
        nc.tensor.matmul(psb[:], ones1[:], bv_sb[:], start=True, stop=True)
        nc.scalar.copy(bv256[:], psb[:])
        for j in range(NCHUNK):
            ps = psAB.tile([128, C], F32, tag="vps", name="vps")
            nc.tensor.matmul(ps[:], xT_cm[0][:, 128 * j:128 * (j + 1)],
                             wv_sb[:, 0, :], start=True, stop=False)
            nc.tensor.matmul(ps[:], xT_cm[1][:, 128 * j:128 * (j + 1)],
                             wv_sb[:, 1, :], start=False, stop=True)
            nc.vector.tensor_tensor(vT[:, j, :], ps[:], bv256[:], op=ALU.add)
        if debug:
            nc.gpsimd.dma_start(
                d["dbg_vt"].rearrange("(j p) c -> p j c", p=128), vT[:])

        # ---- phase C: attention logits -> exp -> normalize ----
        # AE grid: (g1=u, g2=v); AE[g1+1, g2+1] = softmax-numerator of the
        # ORIGINAL position (h=g2, w=g1) (x transposed-grid ordering).
        cmAB.__exit__(None, None, None)
        cmC = tc.tile_pool(name="psC", bufs=2, space="PSUM"); psC = cmC.__enter__()
        AE = main.tile([81, 66 * 67], BF16, tag="accA", name="AE")
        nc.gpsimd.memset(AE[:], 0.0)
        AE3 = AE.rearrange("p (r s) -> p r s", r=67)

        for n8 in range(8):
            ps = psC.tile([81, 512], F32, tag="aps", name="aps")
            for kc in range(2):
                nc.tensor.matmul(ps[:], wa_sb[:, kc, :],
                                 xT_cm[kc][:, 512 * n8:512 * (n8 + 1)],
                                 start=(kc == 0), stop=(kc == 1))
            nc.scalar.activation(AE3[:, 1 + 8 * n8:1 + 8 * n8 + 8, 1:65],
                                 ps.rearrange("p (r s) -> p r s", s=64),
                                 ACT.Exp, bias=ba_sb[:, 0:1])
        # per-p row sums via selector matmul over contiguous padded windows
        # (junk at pad columns is skipped by the strided views)
        ROWCH = [(r0, min(7, 64 - r0)) for r0 in range(0, 64, 7)]
        for r0, nr in ROWCH:
            N = nr * 66
            win = slice((r0 + 1) * 66, (r0 + 1) * 66 + N)
            ps = psC.tile([9, 512], F32, tag="sps", name="sps")
            nc.tensor.matmul(ps[:, 0:N], selsum[:], AE[:, win],
                             start=True, stop=True)
            rchf = consts.tile([9, 512], F32, tag="rchunkf", name="rchf", bufs=1)
            nc.vector.reciprocal_approx_fast(rchf[:, 0:N], ps[:, 0:N])
            rch = consts.tile([9, 512], BF16, tag="rchunk", name="rch", bufs=1)
            nc.scalar.copy(rch[:, 0:N], rchf[:, 0:N])
            ps2 = psC.tile([81, 512], F32, tag="rps", name="rps")
            nc.tensor.matmul(ps2[:, 0:N], selrep_bf[:], rch[:, 0:N],
                             start=True, stop=True)
            iv = AE3[:, r0 + 1:r0 + 1 + nr, 1:65]
            nc.vector.tensor_tensor(
                iv, iv, ps2[:, 0:N].rearrange("p (r s) -> p r s", s=66)[:, :, 1:65],
                op=ALU.mult)
        if debug:
            nc.gpsimd.dma_start(d["dbg_ae"][:], AE[:, 0:4356])

        # ---- phase D: W stencil build (9 shifted selector matmuls) ----
        # Output grid of the windowed matmuls is (g1, g2); the token for
        # (g1, g2) is (u=g2, v=g1), handled by the transpose-scatter evac.
        # Row shift uses dj, column shift di (AE grid is transposed).
        cmC.__exit__(None, None, None)
        cmD = tc.tile_pool(name="psD", bufs=8, space="PSUM"); psD = cmD.__enter__()
        W_tap = main.tile([25, L], F32, tag="s16b", name="W_tap", bufs=2)
        wmask = main.tile([25, L], BF16, tag="s16a", name="wmask")
        nc.gpsimd.dma_start(wmask[:], d["wmask"][:])
        wmask_t = wmask.rearrange("p (u v) -> p v u", u=64)
        wtap_t = W_tap.rearrange("p (u v) -> p v u", u=64)
        for r0, nr in ROWCH:
            N = nr * 66
            ps = psD.tile([25, 512], F32, tag="wps", name="wps")
            for dd, (di, dj) in enumerate(product(range(3), range(3))):
                st = (r0 + 2 - dj) * 66 + (2 - di)
                nc.tensor.matmul(ps[:, 0:N],
                                 selshift[:, 25 * dd:25 * (dd + 1)],
                                 AE[:, st:st + N],
                                 start=(dd == 0), stop=(dd == 8))
            nc.vector.tensor_tensor(
                wtap_t[:, r0:r0 + nr, :],
                ps[:, 0:N].rearrange("p (r s) -> p r s", s=66)[:, :, 0:64],
                wmask_t[:, r0:r0 + nr, :], op=ALU.mult)
        if debug:
            nc.gpsimd.dma_start(d["dbg_w"][:], W_tap[:])
        cmD.__exit__(None, None, None)
        cmD2 = tc.tile_pool(name="psD2", bufs=2, space="PSUM"); psD2 = cmD2.__enter__()
        W_tm = main.tile([128, NCHUNK, 25], F32, tag="W_tm", name="W_tm")
        for j in range(NCHUNK):
            pt = psD2.tile([128, 25], F32, tag="wtp", name="wtp")
            nc.tensor.transpose(pt[:], W_tap[:, 128 * j:128 * (j + 1)],
                                ident[0:25, 0:25])
            nc.scalar.copy(W_tm[:, j, :], pt[:])
        cmD2.__exit__(None, None, None)

        # ---- phase F: maxpools on xT_cm (transposed grid, c-major) ----
        ptmp = es.enter_context(tc.tile_pool(name="ptmp", bufs=3))
        m1 = [main.tile([128, L], BF16, tag=f"s8{'ab'[cc]}", name=f"m1_{cc}") for cc in range(2)]
        m2 = [main.tile([128, L], BF16, tag=["vT", "W_tm"][cc], name=f"m2_{cc}") for cc in range(2)]

        def g3(ap):
            return ap.rearrange("p (h w) -> p h w", h=64)

        def hmax3(eng, dst, src):
            dv, sv = g3(dst), g3(src)
            t1 = ptmp.tile([128, L], BF16, tag="ptmp", name="ptmp")
            tv = g3(t1)
            eng.tensor_tensor(tv[:, :, 1:], sv[:, :, 1:], sv[:, :, :63], op=ALU.max)
            nc.scalar.copy(tv[:, :, 0:1], sv[:, :, 0:1])
            eng.tensor_tensor(dv[:, :, :63], tv[:, :, :63], sv[:, :, 1:], op=ALU.max)
            nc.scalar.copy(dv[:, :, 63:64], tv[:, :, 63:64])

        def vmax3(eng, dst, src):
            dv, sv = g3(dst), g3(src)
            t1 = ptmp.tile([128, L], BF16, tag="ptmp", name="ptmp")
            tv = g3(t1)
            eng.tensor_tensor(tv[:, 1:, :], sv[:, 1:, :], sv[:, :63, :], op=ALU.max)
            nc.scalar.copy(tv[:, 0:1, :], sv[:, 0:1, :])
            eng.tensor_tensor(dv[:, :63, :], tv[:, :63, :], sv[:, 1:, :], op=ALU.max)
            nc.scalar.copy(dv[:, 63:64, :], tv[:, 63:64, :])

        def hspread(eng, dst, src):   # dst[v] = max(src[v-1], src[v+1]) + edge copies
            dv, sv = g3(dst), g3(src)
            eng.tensor_tensor(dv[:, :, 1:63], sv[:, :, 0:62], sv[:, :, 2:64], op=ALU.max)
            nc.scalar.copy(dv[:, :, 0:1], sv[:, :, 1:2])
            nc.scalar.copy(dv[:, :, 63:64], sv[:, :, 62:63])

        def vspread(eng, dst, src):
            dv, sv = g3(dst), g3(src)
            eng.tensor_tensor(dv[:, 1:63, :], sv[:, 0:62, :], sv[:, 2:64, :], op=ALU.max)
            nc.scalar.copy(dv[:, 0:1, :], sv[:, 1:2, :])
            nc.scalar.copy(dv[:, 63:64, :], sv[:, 62:63, :])

        for cc in range(2):
            eng = nc.vector
            cm3 = ptmp.tile([128, L], BF16, tag="ptmp", name="ptmp")
            hmax3(eng, cm3, xT_cm[cc])
            vmax3(eng, m1[cc], cm3)
            cm5 = ptmp.tile([128, L], BF16, tag="ptmp", name="ptmp")
            hspread(eng, cm5, cm3)
            r35 = ptmp.tile([128, L], BF16, tag="ptmp", name="ptmp")
            vmax3(eng, r35, cm5)
            vspread(eng, m2[cc], r35)

        # ---- phase E: 25-tap apply (token-major FMAs, DVE + GPSIMD) ----
        # Constraints: SBUF APs of compute ops must (a) start at partition
        # 0/32/64/96 and (b) use identical partition ranges across operands.
        # So: within-row (f) shifts of vT are pre-materialized via PE
        # shift-matmuls; row shifts (e): even e = chunk offsets (free dim),
        # odd e = accumulate in a 64-token-shifted frame with PE-shifted W,
        # then PE-shift the partial accumulator back and add.
        psE = tc.tile_pool(name="psE", bufs=1, space="PSUM")
        psEp = psE.__enter__()
        acc_d = main.tile([128, NCHUNK, C], F32, tag="accA", name="acc_d")

        def _fma(eng, first, acc, j, srcv, w):
            dst = acc[:, j, :]
            if first:
                eng.tensor_scalar(dst, srcv, w, None, op0=ALU.mult)
            else:
                eng.scalar_tensor_tensor(dst, srcv, w, dst,
                                         op0=ALU.mult, op1=ALU.add)

        SIDX = {d: i for i, d in enumerate(
            (-2, -1, 1, 2, 62, 63, 64, 65, 66, -62, -63, -64, -65, -66))}

        def materialize(delta):
            """vd[token] = vT[token + delta] (zeros out of range), via PE."""
            vd = main.tile([128, NCHUNK, C], BF16, tag="s16b",
                           name=f"vd_{delta}", bufs=2)
            fi = SIDX[delta]
            for j in range(0, NCHUNK, 2):
                ps = psEp.tile([128, 2, C], F32, tag="shps", name="shps",
                               bufs=4)
                j0 = j + (1 if delta > 0 else -1)
                c0, c1 = max(j0, 0), min(j0 + 2, NCHUNK)
                nc.tensor.matmul(ps[:], shifts_sb[:, 2 * fi, :],
                                 vT[:, j:j + 2, :], start=True,
                                 stop=(c1 <= c0))
                if c1 > c0:
                    nc.tensor.matmul(ps[:, c0 - j0:c1 - j0, :],
                                     shifts_sb[:, 2 * fi + 1, :],
                                     vT[:, c0:c1, :], start=False,
                                     stop=True)
                nc.scalar.copy(vd[:, j:j + 2, :], ps[:])
            return vd

        # center tap first: full-coverage init of acc_d
        for j in range(NCHUNK):
            _fma(nc.vector, True, acc_d, j, vT[:, j, :], W_tm[:, j:j + 1, 12:13])

        for f in (0, -2, -1, 1, 2):
            vsrc = vT if f == 0 else materialize(f)
            # even e: chunk offsets on the f-shifted copy
            for e in (-2, 0, 2):
                if (e, f) == (0, 0):
                    continue
                t = (e + 2) * 5 + (f + 2)
                for j in range(NCHUNK):
                    jp = j + e // 2
                    if 0 <= jp < NCHUNK:
                        _fma(nc.vector, False, acc_d, j, vsrc[:, jp, :],
                             W_tm[:, j:j + 1, t:t + 1])
            # odd e: fully shifted copies, direct accumulation
            for e in (1, -1):
                t = (e + 2) * 5 + (f + 2)
                vd = materialize(64 * e + f)
                for j in range(NCHUNK):
                    _fma(nc.vector, False, acc_d, j, vd[:, j, :],
                         W_tm[:, j:j + 1, t:t + 1])
        psE.__exit__(None, None, None)
        # ---- phase G: xf transpose-evac + relu/maxpool chain ----
        # x1 = relu(relu(xfT) + m1^T); x2 = relu(x1 + m2^T)
        cmG = tc.tile_pool(name="psG", bufs=3, space="PSUM"); psG = cmG.__enter__()
        x1 = [main.tile([128, L], BF16, tag=f"s16{'ab'[cc]}", name=f"x1_{cc}",
                        bufs=(2 if cc == 1 else None)) for cc in range(2)]
        x2 = [main.tile([128, L], BF16, tag=f"s8{'ab'[cc]}", name=f"x2_{cc}") for cc in range(2)]
        for j in range(NCHUNK):
            for cc in range(2):
                pt = psG.tile([128, 128], F32, tag="tp", name="tp")
                nc.tensor.transpose(pt[:], acc_d[:, j, 128 * cc:128 * (cc + 1)],
                                    ident[:])
                nc.scalar.activation(x1[cc][:, 128 * j:128 * (j + 1)], pt[:],
                                     ACT.Relu)
        if debug:
            for cc in range(2):
                nc.gpsimd.dma_start(d["dbg_xf"][128 * cc:128 * (cc + 1), :], x1[cc][:])

        for cc in range(2):
            nc.vector.tensor_tensor(x1[cc][:], x1[cc][:], m1[cc][:], op=ALU.add)
            nc.scalar.activation(x1[cc][:], x1[cc][:], ACT.Relu)
            nc.vector.tensor_tensor(x2[cc][:], x1[cc][:], m2[cc][:], op=ALU.add)
            nc.scalar.activation(x2[cc][:], x2[cc][:], ACT.Relu)
        if debug:
            for cc in range(2):
                nc.gpsimd.dma_start(d["dbg_x1"][128 * cc:128 * (cc + 1), :], x1[cc][:])

        # ---- phase H: fu matmul + residual (mc-outer), BN per half ----
        psH = cmH = None
        cmH = tc.tile_pool(name="psH", bufs=4, space="PSUM"); psH = cmH.__enter__()
        out_all = main.tile([128, 2, L], F32, tag="big_a", name="out_all")
        out_cm = [out_all[:, cc, :] for cc in range(2)]
        small = es.enter_context(tc.tile_pool(name="small", bufs=1))
        bnpack = small.tile([128, 4], F32, tag="bnpack", name="bnpack")
        cins = [dram.tile([128, 2], F32, name=f"cin{m}") for m in range(2)]
        couts = [dram.tile([128, 2], F32, name=f"cout{m}") for m in range(2)]
        rhss = [x1[0], x1[1], x2[0], x2[1]]
        for mc in range(2):
            for n8 in range(8):
                sl = slice(512 * n8, 512 * (n8 + 1))
                ps = psH.tile([128, 512], F32, tag="fups", name="fups")
                for kc in range(4):
                    nc.tensor.matmul(ps[:], wfu_sb[:, kc, mc, :],
                                     rhss[kc][:, sl],
                                     start=(kc == 0), stop=(kc == 3))
                nc.scalar.activation(out_cm[mc][:, sl], ps[:], ACT.Relu,
                                     bias=bfu2[:, mc:mc + 1])
                nc.vector.tensor_tensor(out_cm[mc][:, sl], out_cm[mc][:, sl],
                                        xT_cm[mc][:, sl], op=ALU.add)
            # local stats for this half, then its own tiny AllReduce
            st = small.tile([128, 8, 6], F32, tag="bnst", name="bnst")
            for n8 in range(8):
                nc.vector.bn_stats(st[:, n8, :], out_cm[mc][:, 512 * n8:512 * (n8 + 1)])
            ag = small.tile([128, 2], F32, tag="bnag", name="bnag")
            nc.vector.bn_aggr(ag[:], st[:])
            nc.vector.tensor_scalar(bnpack[:, 2 * mc:2 * mc + 1], ag[:, 0:1],
                                    float(L), None, op0=ALU.mult)
            sq = small.tile([128, 1], F32, tag="bnsq", name="bnsq")
            nc.vector.tensor_tensor(sq[:], ag[:, 0:1], ag[:, 0:1], op=ALU.mult)
            nc.vector.tensor_tensor(sq[:], sq[:], ag[:, 1:2], op=ALU.add)
            nc.vector.tensor_scalar(bnpack[:, 2 * mc + 1:2 * mc + 2], sq[:],
                                    float(L), None, op0=ALU.mult)
            nc.sync.dma_start(cins[mc][:], bnpack[:, 2 * mc:2 * mc + 2])
            nc.gpsimd.collective_compute(
                "AllReduce", ALU.add,
                replica_groups=[list(range(n_cores))],
                ins=[cins[mc].opt()], outs=[couts[mc].opt()])
        if debug:
            for cc in range(2):
                nc.sync.dma_start(d["dbg_out"][128 * cc:128 * (cc + 1), :], out_cm[cc][:])
        gs = small.tile([128, 4], F32, tag="gs", name="gs")
        for mc in range(2):
            nc.sync.dma_start(gs[:, 2 * mc:2 * mc + 2], couts[mc][:])
        NTOT = float(n_cores * L)
        scale = small.tile([128, 2], F32, tag="scale", name="scale")
        shift = small.tile([128, 2], F32, tag="shift", name="shift")
        mean = small.tile([128, 2], F32, tag="mean", name="mean")
        var = small.tile([128, 2], F32, tag="var", name="var")
        for cc in range(2):
            nc.vector.tensor_scalar(mean[:, cc:cc + 1], gs[:, 2 * cc:2 * cc + 1],
                                    1.0 / NTOT, None, op0=ALU.mult)
            nc.vector.tensor_scalar(var[:, cc:cc + 1], gs[:, 2 * cc + 1:2 * cc + 2],
                                    1.0 / NTOT, None, op0=ALU.mult)
        msq = small.tile([128, 2], F32, tag="msq", name="msq")
        nc.vector.tensor_tensor(msq[:], mean[:], mean[:], op=ALU.mult)
        nc.vector.tensor_tensor(var[:], var[:], msq[:], op=ALU.subtract)
        rs = small.tile([128, 2], F32, tag="rs", name="rs")
        nc.vector.tensor_scalar(var[:], var[:], float(EPS), None, op0=ALU.add)
        nc.scalar.activation(rs[:], var[:], ACT.Sqrt)
        nc.vector.reciprocal(rs[:], rs[:])
        nc.vector.tensor_tensor(scale[:], gamma2[:], rs[:], op=ALU.mult)
        nc.vector.tensor_tensor(shift[:], mean[:], scale[:], op=ALU.mult)
        nc.vector.tensor_tensor(shift[:], beta2[:], shift[:], op=ALU.subtract)

        # normalize in place, transpose to token-major, DMA out
        cmH.__exit__(None, None, None)
        cmF = tc.tile_pool(name="psF", bufs=3, space="PSUM"); psF = cmF.__enter__()
        ystage = main.tile([128, NCHUNK, C], F32, tag="accA", name="ystage")   # reuse acc_d slot
        for n8 in range(8):
            sl = slice(512 * n8, 512 * (n8 + 1))
            for cc in range(2):
                nc.vector.tensor_scalar(out_cm[cc][:, sl], out_cm[cc][:, sl],
                                        scale[:, cc:cc + 1], shift[:, cc:cc + 1],
                                        op0=ALU.mult, op1=ALU.add)
            for jj in range(4):
                j = 4 * n8 + jj
                for cc in range(2):
                    pt = psF.tile([128, 128], F32, tag="tp", name="tp")
                    nc.tensor.transpose(pt[:], out_cm[cc][:, 128 * j:128 * (j + 1)],
                                        ident[:])
                    nc.scalar.copy(ystage[:, j, 128 * cc:128 * (cc + 1)], pt[:])
        yview = d["y"].rearrange("(j p) c -> p j c", p=128)
        for n8 in range(8):
            nc.sync.dma_start(yview[:, 4 * n8:4 * (n8 + 1), :],
                              ystage[:, 4 * n8:4 * (n8 + 1), :])
        cmF.__exit__(None, None, None)


_CACHE = {}


def _get_program(n_cores=N_CORES, debug=False):
    key = (n_cores, debug)
    if key not in _CACHE:
        nc = bacc.Bacc("TRN2", target_bir_lowering=False, debug=False,
                       num_devices=n_cores)
        build(nc, n_cores, debug)
        nc.compile()
        _CACHE[key] = nc
    return _CACHE[key]


def make_in_map(inputs, b):
    consts = host_consts()
    import ml_dtypes
    xbf = np.ascontiguousarray(inputs["x"][b].reshape(L, C)).astype(ml_dtypes.bfloat16)
    return {
        "xbf": xbf,
        "wv": np.ascontiguousarray(inputs["Wv"], np.float32),
        "bv": np.ascontiguousarray(inputs["bv"].reshape(1, C), np.float32),
        "wa": np.ascontiguousarray(inputs["Wa"], np.float32),
        "ba": np.ascontiguousarray(inputs["ba"].reshape(81, 1), np.float32),
        "wfu": np.ascontiguousarray(inputs["Wfu"], np.float32),
        "bfu2": np.ascontiguousarray(
            inputs["bfu"].reshape(2, 128).T, np.float32),
        "gamma2": np.ascontiguousarray(
            inputs["gamma"].reshape(2, 128).T, np.float32),
        "beta2": np.ascontiguousarray(
            inputs["beta"].reshape(2, 128).T, np.float32),
        **consts,
    }


def kernel(**inputs):
    nc = _get_program()
    in_maps = [make_in_map(inputs, b) for b in range(B)]
    res = run_bass_kernel_spmd(nc, in_maps, list(range(N_CORES)))
    out = np.stack([res.results[b]["y"].reshape(H, W, C) for b in range(B)])
    return out.astype(np.float32)



# revision 8
# speedup vs baseline: 1.3738x; 1.3738x over previous
"""Trainium2 Bass kernel for nn_MOA_13254269075617 (sparse windowed attention block).

Sharding: data-parallel over batch B=8 across 8 NeuronCores (1 image each).
BatchNorm uses global batch stats via an on-device AllReduce of per-channel
sum / sum-of-squares.

Per-core pipeline (all in the spatially-TRANSPOSED frame; host pre-permutes
the input to token order l' = w*64 + h and un-permutes the c-major output):
  xT_cm  : x channel-major [2x128, 4096] via transpose-DMAs
  vT     : (x @ Wv + bv) token-major [128, 32, 256] bf16
  A      : softmax(x @ Wa + ba) pq-major [81, 4096] on a zero-padded grid
  W_tap  : 25-tap position-varying stencil weights [25, 4096] (fold+attention
           combined algebraically), wmask'd at grid edges
  G      : banded token->token weight matrices assembled in DRAM by strided
           scatter-DMA (5-tap f-runs = 10B descriptors), stored transposed
           [m, k] and un-transposed by the DMA XBAR on load
  xf     : stencil apply = 6 PE matmuls per 128-token chunk,
           out[c, m] = sum_k vT[k, c] * G[k, m], PSUM-accumulated c-major
  x1/x2  : relu chains with 3x3/5x5 maxpools (separable shifted-max trees)
  out    : concat-matmul (Wfu) + residual, BN with AllReduce'd stats,
           written c-major [256, 4096]; host transposes back
"""
import sys

for _p in (
    "/root/.axon_site",
    "/root/.axon_site/_ro/trn_rl_repo",
    "/root/.axon_site/_ro/pypackages",
    "/opt/trn_rl_repo",
):
    if _p not in sys.path:
        sys.path.append(_p)

from itertools import product

import numpy as np

import concourse.bass as bass
import concourse.tile as tile
from concourse.ap import AP
from concourse import bacc, mybir
from concourse.bass_utils import run_bass_kernel_spmd

F32 = mybir.dt.float32
BF16 = mybir.dt.bfloat16
ALU = mybir.AluOpType
ACT = mybir.ActivationFunctionType

B, H, W, C = 8, 64, 64, 256
L = H * W                      # 4096 tokens
NCHUNK = L // 128              # 32 token chunks
N_CORES = 8
EPS = 1e-5
GROW = 384                     # 3 source blocks x 128 rows per chunk
GJ = GROW * 128                # G elems per chunk
GSPAD = NCHUNK * GJ + GJ + 4096   # guard for conservative OOB checks
TAPS = [(e, f) for e in range(-2, 3) for f in range(-2, 3)]


def host_consts():
    """Selector matrices and small constants (host-precomputed, same all cores)."""
    selsum = np.zeros((81, 9), np.float32)
    for p in range(9):
        selsum[9 * p:9 * p + 9, p] = 1.0
    selrep = np.zeros((9, 81), np.float32)
    for p in range(9):
        selrep[p, 9 * p:9 * p + 9] = 1.0
    # selshift[:, 25*d + tap]: for (di,dj) block d, tap (e,f):
    #   k = 9*(3di+dj) + 3(di+e)+(dj+f) if di+e,dj+f in [0,3)
    selshift = np.zeros((81, 9 * 25), np.float32)
    for d, (di, dj) in enumerate(product(range(3), range(3))):
        for t, (e, f) in enumerate(product(range(-2, 3), range(-2, 3))):
            dip, djp = di + e, dj + f
            if 0 <= dip < 3 and 0 <= djp < 3:
                k = 9 * (3 * di + dj) + (3 * dip + djp)
                selshift[k, 25 * d + t] = 1.0
    wmask = np.ones((25, 64, 64), np.float32)
    for t, (e, f) in enumerate(product(range(-2, 3), range(-2, 3))):
        if e > 0: wmask[t, 64 - e:, :] = 0
        if e < 0: wmask[t, :-e, :] = 0
        if f > 0: wmask[t, :, 64 - f:] = 0
        if f < 0: wmask[t, :, :-f] = 0
    import ml_dtypes
    return {
        "selsum": selsum,
        "selrep": selrep,
        "selshift": selshift,
        "wmask": wmask.reshape(25, 4096),
        "ident25": np.eye(25, dtype=np.float32),
        "ones1": np.ones((1, 128), np.float32),
        "gs": np.zeros(GSPAD, dtype=ml_dtypes.bfloat16),
    }


def build(nc, n_cores):
    d = {}
    def din(name, shape):
        d[name] = nc.dram_tensor(name, list(shape), F32, kind="ExternalInput").ap()

    d["xbf"] = nc.dram_tensor("xbf", [L, C], BF16, kind="ExternalInput").ap()
    d["gs"] = nc.dram_tensor("gs", [GSPAD], BF16, kind="ExternalInput").ap()
    d["wtmd"] = nc.dram_tensor("wtmd", [L * 25], BF16, kind="Internal").ap()
    din("wv", (C, C)); din("bv", (1, C))
    din("wa", (C, 81)); din("ba", (81, 1))
    din("wfu", (2 * C, C)); din("bfu2", (128, 2))
    din("gamma2", (128, 2)); din("beta2", (128, 2))
    din("selsum", (81, 9)); din("selrep", (9, 81)); din("selshift", (81, 225))
    din("ident25", (25, 25)); din("ones1", (1, 128)); din("wmask", (25, L))
    d["y"] = nc.dram_tensor("y", [2 * 128, L], F32, kind="ExternalOutput").ap()

    with tile.TileContext(nc) as tc:
        _build_tc(tc, d, n_cores)
    return d


def _build_tc(tc, d, n_cores):
    nc = tc.nc
    from contextlib import ExitStack
    es = ExitStack()
    with es:
        consts = es.enter_context(tc.tile_pool(name="consts", bufs=1))
        main = es.enter_context(tc.tile_pool(name="main", bufs=1))
        gpool = es.enter_context(tc.tile_pool(name="gpool", bufs=8))
        dram = es.enter_context(tc.tile_pool(name="dram", bufs=2, space="DRAM"))

        # ---- const loads ----
        def cload(name, shape):
            t = consts.tile(list(shape), F32, tag=name, name=name)
            nc.sync.dma_start(t[:], d[name][:])
            return t
        def cload_bf(name, shape):
            t = consts.tile(list(shape), BF16, tag=name, name=name)
            nc.gpsimd.dma_start(t[:], d[name][:])
            return t
        ones1 = cload_bf("ones1", (1, 128))
        ident25 = cload_bf("ident25", (25, 25))
        bv_sb = cload_bf("bv", (1, C))
        ba_sb = cload("ba", (81, 1))
        selsum = cload_bf("selsum", (81, 9))
        selrep_bf = cload_bf("selrep", (9, 81))
        selshift = cload_bf("selshift", (81, 225))
        bfu2 = cload("bfu2", (128, 2))
        gamma2 = cload("gamma2", (128, 2))
        beta2 = cload("beta2", (128, 2))
        wv_sb = consts.tile([128, 2, C], BF16, tag="wv", name="wv_sb")
        for kc in range(2):
            nc.gpsimd.dma_start(wv_sb[:, kc, :], d["wv"][128 * kc:128 * (kc + 1), :])
        wa_sb = consts.tile([128, 2, 81], BF16, tag="wa", name="wa_sb")
        for kc in range(2):
            nc.gpsimd.dma_start(wa_sb[:, kc, :], d["wa"][128 * kc:128 * (kc + 1), :])
        wfu_sb = consts.tile([128, 4, 2, 128], BF16, tag="wfu", name="wfu_sb")
        for kc in range(4):
            for mc in range(2):
                nc.gpsimd.dma_start(
                    wfu_sb[:, kc, mc, :],
                    d["wfu"][128 * kc:128 * (kc + 1), 128 * mc:128 * (mc + 1)])
        wmask = main.tile([25, L], BF16, tag="wmask", name="wmask")
        nc.gpsimd.dma_start(wmask[:], d["wmask"][:])

        # ---- phase A: transpose-DMA x straight into channel-major ----
        # host pre-permuted xbf rows to l' = w*64 + h, so no on-chip permute
        xT_cm = [main.tile([128, L], BF16, tag=f"xcm{cc}", name=f"xT_cm{cc}")
                 for cc in range(2)]
        for cc in range(2):
            for q in range(8):
                eng = nc.sync if (q % 2 == 0) else nc.scalar
                eng.dma_start_transpose(
                    xT_cm[cc][:, 512 * q:512 * (q + 1)],
                    d["xbf"][512 * q:512 * (q + 1), 128 * cc:128 * (cc + 1)])

        # ---- phase B: vT = xT @ Wv + bv, token-major (bf16) ----
        cmAB = tc.tile_pool(name="psAB", bufs=3, space="PSUM"); psAB = cmAB.__enter__()
        vT = main.tile([128, NCHUNK, C], BF16, tag="vT", name="vT")
        bv256 = consts.tile([128, C], BF16, tag="bv256", name="bv256")
        psb = psAB.tile([128, C], F32, tag="vps", name="vps")
        nc.tensor.matmul(psb[:], ones1[:], bv_sb[:], start=True, stop=True)
        nc.scalar.copy(bv256[:], psb[:])
        for j in range(NCHUNK):
            ps = psAB.tile([128, C], F32, tag="vps", name="vps")
            nc.tensor.matmul(ps[:], xT_cm[0][:, 128 * j:128 * (j + 1)],
                             wv_sb[:, 0, :], start=True, stop=False)
            nc.tensor.matmul(ps[:], xT_cm[1][:, 128 * j:128 * (j + 1)],
                             wv_sb[:, 1, :], start=False, stop=True)
            nc.vector.tensor_tensor(vT[:, j, :], ps[:], bv256[:], op=ALU.add)

        # ---- phase C: attention logits -> exp -> normalize ----
        cmAB.__exit__(None, None, None)
        cmC = tc.tile_pool(name="psC", bufs=2, space="PSUM"); psC = cmC.__enter__()
        AE = main.tile([81, 66 * 67], BF16, tag="AE", name="AE")
        nc.gpsimd.memset(AE[:], 0.0)
        AE3 = AE.rearrange("p (r s) -> p r s", r=67)

        for n8 in range(8):
            ps = psC.tile([81, 512], F32, tag="aps", name="aps")
            for kc in range(2):
                nc.tensor.matmul(ps[:], wa_sb[:, kc, :],
                                 xT_cm[kc][:, 512 * n8:512 * (n8 + 1)],
                                 start=(kc == 0), stop=(kc == 1))
            nc.scalar.activation(AE3[:, 1 + 8 * n8:1 + 8 * n8 + 8, 1:65],
                                 ps.rearrange("p (r s) -> p r s", s=64),
                                 ACT.Exp, bias=ba_sb[:, 0:1])
        ROWCH = [(r0, min(7, 64 - r0)) for r0 in range(0, 64, 7)]
        for r0, nr in ROWCH:
            N = nr * 66
            win = slice((r0 + 1) * 66, (r0 + 1) * 66 + N)
            ps = psC.tile([9, 512], F32, tag="sps", name="sps")
            nc.tensor.matmul(ps[:, 0:N], selsum[:], AE[:, win],
                             start=True, stop=True)
            rchf = consts.tile([9, 512], F32, tag="rchunkf", name="rchf", bufs=1)
            nc.vector.reciprocal_approx_fast(rchf[:, 0:N], ps[:, 0:N])
            rch = consts.tile([9, 512], BF16, tag="rchunk", name="rch", bufs=1)
            nc.scalar.copy(rch[:, 0:N], rchf[:, 0:N])
            ps2 = psC.tile([81, 512], F32, tag="rps", name="rps")
            nc.tensor.matmul(ps2[:, 0:N], selrep_bf[:], rch[:, 0:N],
                             start=True, stop=True)
            iv = AE3[:, r0 + 1:r0 + 1 + nr, 1:65]
            nc.vector.tensor_tensor(
                iv, iv, ps2[:, 0:N].rearrange("p (r s) -> p r s", s=66)[:, :, 1:65],
                op=ALU.mult)

        # ---- phase D: W stencil build (9 shifted selector matmuls) ----
        cmC.__exit__(None, None, None)
        cmD = tc.tile_pool(name="psD", bufs=8, space="PSUM"); psD = cmD.__enter__()
        W_tap = main.tile([25, L], BF16, tag="wtap", name="W_tap")
        wmask_t = wmask.rearrange("p (u v) -> p v u", u=64)
        wtap_t = W_tap.rearrange("p (u v) -> p v u", u=64)
        for r0, nr in ROWCH:
            N = nr * 66
            ps = psD.tile([25, 512], F32, tag="wps", name="wps")
            for dd, (di, dj) in enumerate(product(range(3), range(3))):
                st = (r0 + 2 - dj) * 66 + (2 - di)
                nc.tensor.matmul(ps[:, 0:N],
                                 selshift[:, 25 * dd:25 * (dd + 1)],
                                 AE[:, st:st + N],
                                 start=(dd == 0), stop=(dd == 8))
            nc.vector.tensor_tensor(
                wtap_t[:, r0:r0 + nr, :],
                ps[:, 0:N].rearrange("p (r s) -> p r s", s=66)[:, :, 0:64],
                wmask_t[:, r0:r0 + nr, :], op=ALU.mult)
        cmD.__exit__(None, None, None)

        # ---- phase D2: transpose W to token-major, store to DRAM ----
        cmD2 = tc.tile_pool(name="psD2", bufs=3, space="PSUM"); psD2 = cmD2.__enter__()
        W_tm = main.tile([128, NCHUNK, 25], BF16, tag="wtm", name="W_tm")
        for j in range(NCHUNK):
            pt = psD2.tile([128, 25], BF16, tag="wtp", name="wtp")
            nc.tensor.transpose(pt[:], W_tap[:, 128 * j:128 * (j + 1)],
                                ident25[:])
            nc.scalar.copy(W_tm[:, j, :], pt[:])
        cmD2.__exit__(None, None, None)
        # wtmd[l*25 + t] = W_tm[l%128, l//128, t]
        wtmd_t = d["wtmd"].tensor
        dst = AP(tensor=wtmd_t, offset=0,
                 ap=[[25, 128], [25 * 128, NCHUNK], [1, 25]])
        nc.sync.dma_start(dst, W_tm[:])

        # ---- scatter W into banded G^T in DRAM ----
        # GT[j][m, k] = weight linking source token 128*(j + k//128 - 1) + k%128
        # to output token 128*j + m; tap (e,f) occupies k = m + 64e + f + 128.
        # Flat: gs[j*GJ + 385*m + 64e + f + 128]; the 5-tap f-run is contiguous
        # (10B descriptors). Source is token-major wtmd (50B runs).
        gs_t = d["gs"].tensor
        engs = [nc.sync, nc.scalar]
        ei = 0
        def next_eng():
            nonlocal ei
            ei += 1
            return engs[ei % 2]
        JG = 8                      # j-group size for scatter pipelining
        with nc.allow_non_contiguous_dma(reason="banded G edge diagonals"):
            for e in range(-2, 3):
                rng_f = []
                for f in range(-2, 3):
                    delta = 64 * e + f
                    rng_f.append((max(0, -delta - 128), min(128, 256 - delta)))
                mlo_g = max(r[0] for r in rng_f)
                mhi_g = min(r[1] for r in rng_f)
                for jg in range(0, NCHUNK, JG):
                    dst = AP(tensor=gs_t,
                             offset=jg * GJ + 385 * mlo_g + 64 * e + 126,
                             ap=[[GJ, JG], [385, mhi_g - mlo_g], [1, 5]])
                    src = AP(tensor=wtmd_t,
                             offset=(128 * jg + mlo_g) * 25 + 5 * (e + 2),
                             ap=[[128 * 25, JG], [25, mhi_g - mlo_g], [1, 5]])
                    next_eng().dma_start(dst, src)
                # per-f edge rows clipped out of the group range
                for fi, f in enumerate(range(-2, 3)):
                    lo, hi = rng_f[fi]
                    for m0, m1 in ((lo, mlo_g), (mhi_g, hi)):
                        if m1 <= m0:
                            continue
                        delta = 64 * e + f
                        dst = AP(tensor=gs_t,
                                 offset=385 * m0 + delta + 128,
                                 ap=[[GJ, NCHUNK], [385, m1 - m0], [1, 1]])
                        src = AP(tensor=wtmd_t,
                                 offset=m0 * 25 + 5 * (e + 2) + (f + 2),
                                 ap=[[128 * 25, NCHUNK], [25, m1 - m0], [1, 1]])
                        next_eng().dma_start(dst, src)

        # ---- phase F: maxpools on xT_cm (channel-major grid) ----
        ptmp = es.enter_context(tc.tile_pool(name="ptmp", bufs=3))
        m1 = [main.tile([128, L], BF16, tag=f"m1{cc}", name=f"m1_{cc}") for cc in range(2)]
        m2 = [main.tile([128, L], BF16, tag=f"m2{cc}", name=f"m2_{cc}") for cc in range(2)]

        def g3(ap):
            return ap.rearrange("p (h w) -> p h w", h=64)

        def hmax3(eng, dst, src):
            dv, sv = g3(dst), g3(src)
            t1 = ptmp.tile([128, L], BF16, tag="ptmp", name="ptmp")
            tv = g3(t1)
            eng.tensor_tensor(tv[:, :, 1:], sv[:, :, 1:], sv[:, :, :63], op=ALU.max)
            nc.scalar.copy(tv[:, :, 0:1], sv[:, :, 0:1])
            eng.tensor_tensor(dv[:, :, :63], tv[:, :, :63], sv[:, :, 1:], op=ALU.max)
            nc.scalar.copy(dv[:, :, 63:64], tv[:, :, 63:64])

        def vmax3(eng, dst, src):
            dv, sv = g3(dst), g3(src)
            t1 = ptmp.tile([128, L], BF16, tag="ptmp", name="ptmp")
            tv = g3(t1)
            eng.tensor_tensor(tv[:, 1:, :], sv[:, 1:, :], sv[:, :63, :], op=ALU.max)
            nc.scalar.copy(tv[:, 0:1, :], sv[:, 0:1, :])
            eng.tensor_tensor(dv[:, :63, :], tv[:, :63, :], sv[:, 1:, :], op=ALU.max)
            nc.scalar.copy(dv[:, 63:64, :], tv[:, 63:64, :])

        def hspread(eng, dst, src):   # dst[v] = max(src[v-1], src[v+1]) + edge copies
            dv, sv = g3(dst), g3(src)
            eng.tensor_tensor(dv[:, :, 1:63], sv[:, :, 0:62], sv[:, :, 2:64], op=ALU.max)
            nc.scalar.copy(dv[:, :, 0:1], sv[:, :, 1:2])
            nc.scalar.copy(dv[:, :, 63:64], sv[:, :, 62:63])

        def vspread(eng, dst, src):
            dv, sv = g3(dst), g3(src)
            eng.tensor_tensor(dv[:, 1:63, :], sv[:, 0:62, :], sv[:, 2:64, :], op=ALU.max)
            nc.scalar.copy(dv[:, 0:1, :], sv[:, 1:2, :])
            nc.scalar.copy(dv[:, 63:64, :], sv[:, 62:63, :])

        for cc in range(2):
            eng = nc.vector
            cm3 = ptmp.tile([128, L], BF16, tag="ptmp", name="ptmp")
            hmax3(eng, cm3, xT_cm[cc])
            vmax3(eng, m1[cc], cm3)
            cm5 = ptmp.tile([128, L], BF16, tag="ptmp", name="ptmp")
            hspread(eng, cm5, cm3)
            r35 = ptmp.tile([128, L], BF16, tag="ptmp", name="ptmp")
            vmax3(eng, r35, cm5)
            vspread(eng, m2[cc], r35)

        # ---- phase E: banded stencil apply, c-major out ----
        cmE = tc.tile_pool(name="psE", bufs=4, space="PSUM"); psE = cmE.__enter__()
        x1 = [main.tile([128, L], BF16, tag=f"x1{cc}", name=f"x1_{cc}") for cc in range(2)]
        x2 = [main.tile([128, L], BF16, tag=f"x2{cc}", name=f"x2_{cc}") for cc in range(2)]
        for j in range(NCHUNK):
            g = gpool.tile([128, 3, 128], BF16, tag="g", name="g")
            bs = [b for b in range(3) if 0 <= j + b - 1 < NCHUNK]
            for b in bs:
                eng = nc.sync if (b % 2 == 0) else nc.scalar
                eng.dma_start_transpose(
                    g[:, b, :],
                    AP(tensor=gs_t, offset=j * GJ + 128 * b,
                       ap=[[384, 128], [1, 128]]))
            psx = psE.tile([128, 2, 128], F32, tag="psx", name="psx")
            for cc in range(2):
                for i, b in enumerate(bs):
                    nc.tensor.matmul(psx[:, cc, :],
                                     vT[:, j + b - 1, 128 * cc:128 * (cc + 1)],
                                     g[:, b, :],
                                     start=(i == 0), stop=(i == len(bs) - 1))
                nc.scalar.activation(x1[cc][:, 128 * j:128 * (j + 1)],
                                     psx[:, cc, :], ACT.Relu)
        cmE.__exit__(None, None, None)

        # ---- phase G tail: x1 = relu(xr + m1); x2 = relu(x1 + m2) ----
        for n8 in range(8):
            sl = slice(512 * n8, 512 * (n8 + 1))
            for cc in range(2):
                nc.vector.tensor_tensor(x1[cc][:, sl], x1[cc][:, sl],
                                        m1[cc][:, sl], op=ALU.add)
                nc.scalar.activation(x1[cc][:, sl], x1[cc][:, sl], ACT.Relu)
                nc.vector.tensor_tensor(x2[cc][:, sl], x1[cc][:, sl],
                                        m2[cc][:, sl], op=ALU.add)
                nc.scalar.activation(x2[cc][:, sl], x2[cc][:, sl], ACT.Relu)

        # ---- phase H: fu matmul + residual (mc-outer), BN per half ----
        cmH = tc.tile_pool(name="psH", bufs=4, space="PSUM"); psH = cmH.__enter__()
        out_all = main.tile([128, 2, L], F32, tag="out", name="out_all")
        out_cm = [out_all[:, cc, :] for cc in range(2)]
        small = es.enter_context(tc.tile_pool(name="small", bufs=1))
        bnpack = small.tile([128, 4], F32, tag="bnpack", name="bnpack")
        cins = [dram.tile([128, 2], F32, name=f"cin{m}") for m in range(2)]
        couts = [dram.tile([128, 2], F32, name=f"cout{m}") for m in range(2)]
        rhss = [x1[0], x1[1], x2[0], x2[1]]
        for mc in range(2):
            for n8 in range(8):
                sl = slice(512 * n8, 512 * (n8 + 1))
                ps = psH.tile([128, 512], F32, tag="fups", name="fups")
                for kc in range(4):
                    nc.tensor.matmul(ps[:], wfu_sb[:, kc, mc, :],
                                     rhss[kc][:, sl],
                                     start=(kc == 0), stop=(kc == 3))
                nc.scalar.activation(out_cm[mc][:, sl], ps[:], ACT.Relu,
                                     bias=bfu2[:, mc:mc + 1])
                nc.vector.tensor_tensor(out_cm[mc][:, sl], out_cm[mc][:, sl],
                                        xT_cm[mc][:, sl], op=ALU.add)
            st = small.tile([128, 8, 6], F32, tag="bnst", name="bnst")
            for n8 in range(8):
                nc.vector.bn_stats(st[:, n8, :], out_cm[mc][:, 512 * n8:512 * (n8 + 1)])
            ag = small.tile([128, 2], F32, tag="bnag", name="bnag")
            nc.vector.bn_aggr(ag[:], st[:])
            nc.vector.tensor_scalar(bnpack[:, 2 * mc:2 * mc + 1], ag[:, 0:1],
                                    float(L), None, op0=ALU.mult)
            sq = small.tile([128, 1], F32, tag="bnsq", name="bnsq")
            nc.vector.tensor_tensor(sq[:], ag[:, 0:1], ag[:, 0:1], op=ALU.mult)
            nc.vector.tensor_tensor(sq[:], sq[:], ag[:, 1:2], op=ALU.add)
            nc.vector.tensor_scalar(bnpack[:, 2 * mc + 1:2 * mc + 2], sq[:],
                                    float(L), None, op0=ALU.mult)
            nc.sync.dma_start(cins[mc][:], bnpack[:, 2 * mc:2 * mc + 2])
            nc.gpsimd.collective_compute(
                "AllReduce", ALU.add,
                replica_groups=[list(range(n_cores))],
                ins=[cins[mc].opt()], outs=[couts[mc].opt()])
        gs_sb = small.tile([128, 4], F32, tag="gsb", name="gs_sb")
        for mc in range(2):
            nc.sync.dma_start(gs_sb[:, 2 * mc:2 * mc + 2], couts[mc][:])
        NTOT = float(n_cores * L)
        scale = small.tile([128, 2], F32, tag="scale", name="scale")
        shift = small.tile([128, 2], F32, tag="shift", name="shift")
        mean = small.tile([128, 2], F32, tag="mean", name="mean")
        var = small.tile([128, 2], F32, tag="var", name="var")
        for cc in range(2):
            nc.vector.tensor_scalar(mean[:, cc:cc + 1], gs_sb[:, 2 * cc:2 * cc + 1],
                                    1.0 / NTOT, None, op0=ALU.mult)
            nc.vector.tensor_scalar(var[:, cc:cc + 1], gs_sb[:, 2 * cc + 1:2 * cc + 2],
                                    1.0 / NTOT, None, op0=ALU.mult)
        msq = small.tile([128, 2], F32, tag="msq", name="msq")
        nc.vector.tensor_tensor(msq[:], mean[:], mean[:], op=ALU.mult)
        nc.vector.tensor_tensor(var[:], var[:], msq[:], op=ALU.subtract)
        rs = small.tile([128, 2], F32, tag="rs", name="rs")
        nc.vector.tensor_scalar(var[:], var[:], float(EPS), None, op0=ALU.add)
        nc.scalar.activation(rs[:], var[:], ACT.Sqrt)
        nc.vector.reciprocal(rs[:], rs[:])
        nc.vector.tensor_tensor(scale[:], gamma2[:], rs[:], op=ALU.mult)
        nc.vector.tensor_tensor(shift[:], mean[:], scale[:], op=ALU.mult)
        nc.vector.tensor_tensor(shift[:], beta2[:], shift[:], op=ALU.subtract)

        # normalize in place, DMA out c-major (host un-transposes)
        for n8 in range(8):
            sl = slice(512 * n8, 512 * (n8 + 1))
            for cc in range(2):
                nc.vector.tensor_scalar(out_cm[cc][:, sl], out_cm[cc][:, sl],
                                        scale[:, cc:cc + 1], shift[:, cc:cc + 1],
                                        op0=ALU.mult, op1=ALU.add)
                eng = nc.sync if (n8 % 2 == 0) else nc.scalar
                eng.dma_start(d["y"][128 * cc:128 * (cc + 1), sl],
                              out_cm[cc][:, sl])
        cmH.__exit__(None, None, None)


_CACHE = {}


def _get_program(n_cores=N_CORES):
    key = n_cores
    if key not in _CACHE:
        nc = bacc.Bacc("TRN2", target_bir_lowering=False, debug=False,
                       num_devices=n_cores)
        build(nc, n_cores)
        nc.compile()
        _CACHE[key] = nc
    return _CACHE[key]


_CONSTS = None


def make_in_map(inputs, b):
    global _CONSTS
    if _CONSTS is None:
        _CONSTS = host_consts()
    import ml_dtypes
    # pre-permute to the transposed-grid token order l' = w*64 + h
    xbf = np.ascontiguousarray(
        np.asarray(inputs["x"][b]).transpose(1, 0, 2).reshape(L, C)
    ).astype(ml_dtypes.bfloat16)
    return {
        "xbf": xbf,
        "wv": np.ascontiguousarray(inputs["Wv"], np.float32),
        "bv": np.ascontiguousarray(np.asarray(inputs["bv"]).reshape(1, C), np.float32),
        "wa": np.ascontiguousarray(inputs["Wa"], np.float32),
        "ba": np.ascontiguousarray(np.asarray(inputs["ba"]).reshape(81, 1), np.float32),
        "wfu": np.ascontiguousarray(inputs["Wfu"], np.float32),
        "bfu2": np.ascontiguousarray(
            np.asarray(inputs["bfu"]).reshape(2, 128).T, np.float32),
        "gamma2": np.ascontiguousarray(
            np.asarray(inputs["gamma"]).reshape(2, 128).T, np.float32),
        "beta2": np.ascontiguousarray(
            np.asarray(inputs["beta"]).reshape(2, 128).T, np.float32),
        **_CONSTS,
    }


def postprocess(yarr):
    """[256, L] c-major, l' = w*64+h  ->  [H, W, C] in the reference frame."""
    return np.asarray(yarr, np.float32).reshape(C, L).T.reshape(H, W, C)


def kernel(**inputs):
    nc = _get_program()
    in_maps = [make_in_map(inputs, b) for b in range(B)]
    res = run_bass_kernel_spmd(nc, in_maps, list(range(N_CORES)))
    out = np.stack([postprocess(res.results[b]["y"]) for b in range(B)])
    return out.astype(np.float32)


# revision 37
# speedup vs baseline: 1.4377x; 1.0465x over previous
"""Trainium2 Bass kernel for nn_MOA_13254269075617 (sparse windowed attention block).

Sharding: data-parallel over batch B=8 across 8 NeuronCores (1 image each).
BatchNorm uses global batch stats via an on-device AllReduce of per-channel
sum / sum-of-squares.

Per-core pipeline (all in the spatially-TRANSPOSED frame; host pre-permutes
the input to token order l' = w*64 + h and un-permutes the c-major output):
  xT_cm  : x channel-major [2x128, 4096] via transpose-DMAs
  vT     : (x @ Wv + bv) token-major [128, 32, 256] bf16
  A      : softmax(x @ Wa + ba) pq-major [81, 4096] on a zero-padded grid
  W_tap  : 25-tap position-varying stencil weights [25, 4096] (fold+attention
           combined algebraically), wmask'd at grid edges
  G      : banded token->token weight matrices assembled in DRAM by strided
           scatter-DMA (5-tap f-runs = 10B descriptors), stored transposed
           [m, k] and un-transposed by the DMA XBAR on load
  xf     : stencil apply = 6 PE matmuls per 128-token chunk,
           out[c, m] = sum_k vT[k, c] * G[k, m], PSUM-accumulated c-major
  x1/x2  : relu chains with 3x3/5x5 maxpools (separable shifted-max trees)
  out    : concat-matmul (Wfu) + residual, BN with AllReduce'd stats,
           written c-major [256, 4096]; host transposes back
"""
import sys

for _p in (
    "/root/.axon_site",
    "/root/.axon_site/_ro/trn_rl_repo",
    "/root/.axon_site/_ro/pypackages",
    "/opt/trn_rl_repo",
):
    if _p not in sys.path:
        sys.path.append(_p)

from itertools import product

import numpy as np

import concourse.bass as bass
import concourse.tile as tile
from concourse.ap import AP
from concourse import bacc, mybir
from concourse.bass_utils import run_bass_kernel_spmd

F32 = mybir.dt.float32
BF16 = mybir.dt.bfloat16
ALU = mybir.AluOpType
ACT = mybir.ActivationFunctionType

B, H, W, C = 8, 64, 64, 256
L = H * W                      # 4096 tokens
NCHUNK = L // 128              # 32 token chunks
N_CORES = 8
EPS = 1e-5
GROW = 384                     # 3 source blocks x 128 rows per chunk
GJ = GROW * 128                # G elems per chunk
GSPAD = NCHUNK * GJ + GJ + 4096   # guard for conservative OOB checks
DEBUG = False
TAPS = [(e, f) for e in range(-2, 3) for f in range(-2, 3)]


def host_consts():
    """Selector matrices and small constants (host-precomputed, same all cores)."""
    selsum = np.zeros((81, 9), np.float32)
    for p in range(9):
        selsum[9 * p:9 * p + 9, p] = 1.0
    selrep = np.zeros((9, 81), np.float32)
    for p in range(9):
        selrep[p, 9 * p:9 * p + 9] = 1.0
    # selshift[:, 25*d + tap]: for (di,dj) block d, tap (e,f):
    #   k = 9*(3di+dj) + 3(di+e)+(dj+f) if di+e,dj+f in [0,3)
    selshift = np.zeros((81, 9 * 25), np.float32)
    for d, (di, dj) in enumerate(product(range(3), range(3))):
        for t, (e, f) in enumerate(product(range(-2, 3), range(-2, 3))):
            dip, djp = di + e, dj + f
            if 0 <= dip < 3 and 0 <= djp < 3:
                k = 9 * (3 * di + dj) + (3 * dip + djp)
                selshift[k, 25 * d + t] = 1.0
    wmask = np.ones((25, 64, 64), np.float32)
    for t, (e, f) in enumerate(product(range(-2, 3), range(-2, 3))):
        if e > 0: wmask[t, 64 - e:, :] = 0
        if e < 0: wmask[t, :-e, :] = 0
        if f > 0: wmask[t, :, 64 - f:] = 0
        if f < 0: wmask[t, :, :-f] = 0
    import ml_dtypes
    return {
        "selsum": selsum,
        "selrep": selrep,
        "selshift": selshift,
        "wmask": wmask.reshape(25, 4096),
        "ident25": np.eye(25, dtype=np.float32),
        "ones1": np.ones((1, 128), np.float32),
        "gs": np.zeros(GSPAD, dtype=ml_dtypes.bfloat16),
    }


def build(nc, n_cores):
    d = {}
    def din(name, shape):
        d[name] = nc.dram_tensor(name, list(shape), F32, kind="ExternalInput").ap()

    d["xbf"] = nc.dram_tensor("xbf", [L, C], BF16, kind="ExternalInput").ap()
    d["gs"] = nc.dram_tensor("gs", [GSPAD], BF16, kind="ExternalInput").ap()
    d["wtmd"] = nc.dram_tensor("wtmd", [L * 25], BF16, kind="Internal").ap()
    din("wv", (C, C)); din("bv", (1, C))
    din("wa", (C, 81)); din("ba", (81, 1))
    din("wfu", (2 * C, C)); din("bfu2", (128, 2))
    din("gamma2", (128, 2)); din("beta2", (128, 2))
    din("selsum", (81, 9)); din("selrep", (9, 81)); din("selshift", (81, 225))
    din("ident25", (25, 25)); din("ones1", (1, 128)); din("wmask", (25, L))
    d["y"] = nc.dram_tensor("y", [2 * 128, L], F32, kind="ExternalOutput").ap()
    if DEBUG:
        d["dbg_wtap"] = nc.dram_tensor("dbg_wtap", [25, L], F32, kind="ExternalOutput").ap()
        d["dbg_wtm"] = nc.dram_tensor("dbg_wtm", [128, NCHUNK * 25], F32, kind="ExternalOutput").ap()
        d["dbg_wtmd"] = nc.dram_tensor("dbg_wtmd", [L * 25], BF16, kind="ExternalOutput").ap()
        d["dbg_gs"] = nc.dram_tensor("dbg_gs", [4 * GJ], BF16, kind="ExternalOutput").ap()
        d["dbg_x1"] = nc.dram_tensor("dbg_x1", [2 * 128, L], F32, kind="ExternalOutput").ap()
        d["dbg_vt"] = nc.dram_tensor("dbg_vt", [128, NCHUNK * C], F32, kind="ExternalOutput").ap()

    with tile.TileContext(nc) as tc:
        _build_tc(tc, d, n_cores)
    return d


def _build_tc(tc, d, n_cores):
    nc = tc.nc
    from contextlib import ExitStack
    es = ExitStack()
    with es:
        consts = es.enter_context(tc.tile_pool(name="consts", bufs=1))
        main = es.enter_context(tc.tile_pool(name="main", bufs=1))
        gpool = es.enter_context(tc.tile_pool(name="gpool", bufs=8))
        dram = es.enter_context(tc.tile_pool(name="dram", bufs=2, space="DRAM"))

        # ---- const loads ----
        def cload(name, shape):
            t = consts.tile(list(shape), F32, tag=name, name=name)
            nc.sync.dma_start(t[:], d[name][:])
            return t
        def cload_bf(name, shape):
            t = consts.tile(list(shape), BF16, tag=name, name=name)
            nc.gpsimd.dma_start(t[:], d[name][:])
            return t
        ones1 = cload_bf("ones1", (1, 128))
        ident25 = cload_bf("ident25", (25, 25))
        bv_sb = cload_bf("bv", (1, C))
        ba_sb = cload("ba", (81, 1))
        selsum = cload_bf("selsum", (81, 9))
        selrep_bf = cload_bf("selrep", (9, 81))
        selshift = cload_bf("selshift", (81, 225))
        bfu2 = cload("bfu2", (128, 2))
        gamma2 = cload("gamma2", (128, 2))
        beta2 = cload("beta2", (128, 2))
        wv_sb = consts.tile([128, 2, C], BF16, tag="wv", name="wv_sb")
        for kc in range(2):
            nc.gpsimd.dma_start(wv_sb[:, kc, :], d["wv"][128 * kc:128 * (kc + 1), :])
        wa_sb = consts.tile([128, 2, 81], BF16, tag="wa", name="wa_sb")
        for kc in range(2):
            nc.gpsimd.dma_start(wa_sb[:, kc, :], d["wa"][128 * kc:128 * (kc + 1), :])
        wfu_sb = consts.tile([128, 4, 2, 128], BF16, tag="wfu", name="wfu_sb")
        for kc in range(4):
            for mc in range(2):
                nc.gpsimd.dma_start(
                    wfu_sb[:, kc, mc, :],
                    d["wfu"][128 * kc:128 * (kc + 1), 128 * mc:128 * (mc + 1)])
        wmask = main.tile([25, L], BF16, tag="wmask", name="wmask")
        nc.gpsimd.dma_start(wmask[:], d["wmask"][:])

        # ---- phase A: transpose-DMA x straight into channel-major ----
        # host pre-permuted xbf rows to l' = w*64 + h, so no on-chip permute
        xT_cm = [main.tile([128, L], BF16, tag=f"xcm{cc}", name=f"xT_cm{cc}")
                 for cc in range(2)]
        a_dmas = []
        for cc in range(2):
            for q in range(8):
                eng = nc.sync if (q % 2 == 0) else nc.scalar
                a_dmas.append(eng.dma_start_transpose(
                    xT_cm[cc][:, 512 * q:512 * (q + 1)],
                    d["xbf"][512 * q:512 * (q + 1), 128 * cc:128 * (cc + 1)]))

        # ---- phase B: vT = xT @ Wv + bv, token-major (bf16) ----
        cmAB = tc.tile_pool(name="psAB", bufs=3, space="PSUM"); psAB = cmAB.__enter__()
        vT = main.tile([128, NCHUNK, C], BF16, tag="vT", name="vT")
        bv256 = consts.tile([128, C], BF16, tag="bv256", name="bv256")
        psb = psAB.tile([128, C], F32, tag="vps", name="vps")
        nc.tensor.matmul(psb[:], ones1[:], bv_sb[:], start=True, stop=True)
        nc.scalar.copy(bv256[:], psb[:])
        # PE LDWEIGHTS prefetch hazard: B's stationaries are xT_cm chunks, and
        # LDW precedes the dep-gated matmul in stream order. Gate a PE nop on
        # the phase-A DMAs so no LDW can read xT_cm early. (B is off the
        # critical path: C reads xT_cm directly.)
        from concourse.tile import add_dep_helper as _adh
        bprobe = nc.vector.memset(bv256[:, 0:1].bitcast(F32), 0.0) if False else None
        # PE LDW-prefetch guard: gate a tiny PE matmul (reusing the bv256 one
        # above is not possible), instead gate via vector probe consumed by PE
        guard_t = consts.tile([1, 4], F32, tag="guard", name="guard")
        gprobe = nc.vector.memset(guard_t[:, 0:1], 0.0)
        for dma in a_dmas:
            _adh(gprobe.ins, dma.ins, reason="B LDW prefetch guard")
        for j in range(NCHUNK):
            ps = psAB.tile([128, C], F32, tag="vps", name="vps")
            mmb = nc.tensor.matmul(ps[:], xT_cm[0][:, 128 * j:128 * (j + 1)],
                                   wv_sb[:, 0, :], start=True, stop=False)
            if j == 0:
                _adh(mmb.ins, gprobe.ins, reason="B LDW prefetch guard")
            nc.tensor.matmul(ps[:], xT_cm[1][:, 128 * j:128 * (j + 1)],
                             wv_sb[:, 1, :], start=False, stop=True)
            nc.vector.tensor_tensor(vT[:, j, :], ps[:], bv256[:], op=ALU.add)

        # ---- phase C: attention logits -> exp -> normalize ----
        cmAB.__exit__(None, None, None)
        cmC = tc.tile_pool(name="psC", bufs=2, space="PSUM"); psC = cmC.__enter__()
        AE = main.tile([81, 66 * 67], BF16, tag="AE", name="AE")
        nc.gpsimd.memset(AE[:], 0.0)
        AE3 = AE.rearrange("p (r s) -> p r s", r=67)

        # NOTE: the automatic dependency tracker is unreliable for the strided
        # AE3 views, so cross-engine RAW edges here are added explicitly
        # (per-engine in-order execution covers the downstream instructions).
        from concourse.tile import add_dep_helper
        exp_insts = []
        for n8 in range(8):
            ps = psC.tile([81, 512], F32, tag="aps", name="aps")
            for kc in range(2):
                nc.tensor.matmul(ps[:], wa_sb[:, kc, :],
                                 xT_cm[kc][:, 512 * n8:512 * (n8 + 1)],
                                 start=(kc == 0), stop=(kc == 1))
            exp_insts.append(nc.scalar.activation(
                AE3[:, 1 + 8 * n8:1 + 8 * n8 + 8, 1:65],
                ps.rearrange("p (r s) -> p r s", s=64),
                ACT.Exp, bias=ba_sb[:, 0:1]))
        ROWCH = [(r0, min(7, 64 - r0)) for r0 in range(0, 64, 7)]
        norm_insts = []
        for r0, nr in ROWCH:
            N = nr * 66
            win = slice((r0 + 1) * 66, (r0 + 1) * 66 + N)
            ps = psC.tile([9, 512], F32, tag="sps", name="sps")
            mm = nc.tensor.matmul(ps[:, 0:N], selsum[:], AE[:, win],
                                  start=True, stop=True)
            # rowsum reads AE rows [r0+1, r0+1+nr): wait for the exp blocks
            for n8 in range(max(0, r0 // 8), min(8, (r0 + nr) // 8 + 1)):
                add_dep_helper(mm.ins, exp_insts[n8].ins,
                               reason="rowsum reads exp'd AE rows")
            rchf = consts.tile([9, 512], F32, tag="rchunkf", name="rchf", bufs=1)
            nc.vector.reciprocal_approx_fast(rchf[:, 0:N], ps[:, 0:N])
            rch = consts.tile([9, 512], BF16, tag="rchunk", name="rch", bufs=1)
            nc.scalar.copy(rch[:, 0:N], rchf[:, 0:N])
            ps2 = psC.tile([81, 512], F32, tag="rps", name="rps")
            nc.tensor.matmul(ps2[:, 0:N], selrep_bf[:], rch[:, 0:N],
                             start=True, stop=True)
            iv = AE3[:, r0 + 1:r0 + 1 + nr, 1:65]
            nv = nc.vector.tensor_tensor(
                iv, iv, ps2[:, 0:N].rearrange("p (r s) -> p r s", s=66)[:, :, 1:65],
                op=ALU.mult)
            for n8 in range(max(0, r0 // 8), min(8, (r0 + nr) // 8 + 1)):
                add_dep_helper(nv.ins, exp_insts[n8].ins,
                               reason="normalize RMWs exp'd AE rows")
            norm_insts.append(nv)

        # ---- phase D: W stencil build (9 shifted selector matmuls) ----
        cmC.__exit__(None, None, None)
        cmD = tc.tile_pool(name="psD", bufs=8, space="PSUM"); psD = cmD.__enter__()
        W_tap = main.tile([25, L], BF16, tag="wtap", name="W_tap")
        wmask_t = wmask.rearrange("p (u v) -> p v u", u=64)
        wtap_t = W_tap.rearrange("p (u v) -> p v u", u=64)
        ev_insts = []
        first_d = True
        for r0, nr in ROWCH:
            N = nr * 66
            ps = psD.tile([25, 512], F32, tag="wps", name="wps")
            for dd, (di, dj) in enumerate(product(range(3), range(3))):
                st = (r0 + 2 - dj) * 66 + (2 - di)
                mm = nc.tensor.matmul(ps[:, 0:N],
                                      selshift[:, 25 * dd:25 * (dd + 1)],
                                      AE[:, st:st + N],
                                      start=(dd == 0), stop=(dd == 8))
                if first_d:
                    # PE is in-order: gating the first D matmul on all
                    # normalizes covers every later AE read in phase D
                    for nv in norm_insts:
                        add_dep_helper(mm.ins, nv.ins,
                                       reason="D reads normalized AE")
                    first_d = False
            ev_insts.append(nc.vector.tensor_tensor(
                wtap_t[:, r0:r0 + nr, :],
                ps[:, 0:N].rearrange("p (r s) -> p r s", s=66)[:, :, 0:64],
                wmask_t[:, r0:r0 + nr, :], op=ALU.mult))
        cmD.__exit__(None, None, None)

        # ---- phase D2: transpose W to token-major, store to DRAM ----
        cmD2 = tc.tile_pool(name="psD2", bufs=3, space="PSUM"); psD2 = cmD2.__enter__()
        W_tm = main.tile([128, NCHUNK, 25], BF16, tag="wtm", name="W_tm")
        # D2's stationary is W_tap itself; gate a PE nop on the evacs so the
        # LDWEIGHTS (which precedes the matmul in stream order and can be
        # pulled ahead by the PE queue) cannot read stale W_tap
        if DEBUG:
            nc.gpsimd.dma_start(d["dbg_wtap"][:], W_tap[:])
        for j in range(NCHUNK):
            pt = psD2.tile([128, 25], BF16, tag="wtp", name="wtp")
            tr = nc.tensor.transpose(pt[:], W_tap[:, 128 * j:128 * (j + 1)],
                                     ident25[:])
            if j == 0:
                for ev in ev_insts:
                    add_dep_helper(tr.ins, ev.ins, reason="D2 reads W_tap")
            nc.scalar.copy(W_tm[:, j, :], pt[:])
        cmD2.__exit__(None, None, None)
        # wtmd[l*25 + t] = W_tm[l%128, l//128, t]
        # NOTE: the automatic dependency tracker under-computes the ranges of
        # these exotic strided DRAM APs, so the store->scatter->load edges are
        # added explicitly (lowered by tile into its normal semaphores).
        from concourse.tile import add_dep_helper
        wtmd_t = d["wtmd"].tensor
        dst = AP(tensor=wtmd_t, offset=0,
                 ap=[[25, 128], [25 * 128, NCHUNK], [1, 25]])
        wstore = nc.sync.dma_start(dst, W_tm[:])
        if DEBUG:
            nc.gpsimd.dma_start(
                d["dbg_wtm"][:].rearrange("p (j t) -> p j t", t=25), W_tm[:])

        # ---- scatter W into banded G^T in DRAM ----
        # GT[j][m, k] = weight linking source token 128*(j + k//128 - 1) + k%128
        # to output token 128*j + m; tap (e,f) occupies k = m + 64e + f + 128.
        # Flat: gs[j*GJ + 385*m + 64e + f + 128]; the 5-tap f-run is contiguous
        # (10B descriptors). Source is token-major wtmd (50B runs).
        gs_t = d["gs"].tensor
        engs = [nc.sync, nc.scalar]
        ei = 0
        def next_eng():
            nonlocal ei
            ei += 1
            return engs[ei % 2]
        JG = 16                     # j-group size for scatter pipelining
        GRNG = {}
        for e in range(-2, 3):
            rng_f = []
            for f in range(-2, 3):
                delta = 64 * e + f
                rng_f.append((max(0, -delta - 128), min(128, 256 - delta)))
            GRNG[e] = (max(r[0] for r in rng_f), min(r[1] for r in rng_f), rng_f)
        scat_edge = []              # full-j scatters
        scat_grp = {}               # jg -> list of scatters covering [jg, jg+JG)
        # DMA->DMA dependency edges do not reliably order transfer completion,
        # so route store->scatter->load ordering through engine nops (the
        # DMA->engine and engine->DMA dependency paths are the proven ones).
        probe_t = consts.tile([1, 8], F32, tag="probe", name="probe")
        probe_w = nc.vector.memset(probe_t[:, 0:1], 0.0)
        add_dep_helper(probe_w.ins, wstore.ins, reason="wtmd store complete")
        with nc.allow_non_contiguous_dma(reason="banded G diagonals"):
            # edge rows (clipped out of the f-run groups) first, full-j
            for e in range(-2, 3):
                mlo_g, mhi_g, rng_f = GRNG[e]
                for fi, f in enumerate(range(-2, 3)):
                    lo, hi = rng_f[fi]
                    for m0, m1 in ((lo, mlo_g), (mhi_g, hi)):
                        if m1 <= m0:
                            continue
                        delta = 64 * e + f
                        dst = AP(tensor=gs_t,
                                 offset=385 * m0 + delta + 128,
                                 ap=[[GJ, NCHUNK], [385, m1 - m0], [1, 1]])
                        src = AP(tensor=wtmd_t,
                                 offset=m0 * 25 + 5 * (e + 2) + (f + 2),
                                 ap=[[128 * 25, NCHUNK], [25, m1 - m0], [1, 1]])
                        inst = next_eng().dma_start(dst, src)
                        add_dep_helper(inst.ins, probe_w.ins,
                                       reason="scatter reads wtmd")
                        scat_edge.append(inst)
            # f-run scatters, grouped by j for load pipelining
            for jg in range(0, NCHUNK, JG):
                scat_grp[jg] = []
                for e in range(-2, 3):
                    mlo_g, mhi_g, _ = GRNG[e]
                    dst = AP(tensor=gs_t,
                             offset=jg * GJ + 385 * mlo_g + 64 * e + 126,
                             ap=[[GJ, JG], [385, mhi_g - mlo_g], [1, 5]])
                    src = AP(tensor=wtmd_t,
                             offset=(128 * jg + mlo_g) * 25 + 5 * (e + 2),
                             ap=[[128 * 25, JG], [25, mhi_g - mlo_g], [1, 5]])
                    inst = next_eng().dma_start(dst, src)
                    add_dep_helper(inst.ins, probe_w.ins,
                                   reason="scatter reads wtmd")
                    scat_grp[jg].append(inst)

        if DEBUG:
            dmb = nc.sync.dma_start(d["dbg_wtmd"][:], d["wtmd"][:])
            add_dep_helper(dmb.ins, wstore.ins, reason="dbg")
            dgs = nc.scalar.dma_start(d["dbg_gs"][:], d["gs"][0:4 * GJ])
            for lst in ([*scat_edge] + scat_grp[0]):
                add_dep_helper(dgs.ins, lst.ins, reason="dbg")

        # ---- phase F: maxpools on xT_cm (channel-major grid) ----
        ptmp = es.enter_context(tc.tile_pool(name="ptmp", bufs=3))
        mp_copies = []
        m1 = [main.tile([128, L], BF16, tag=f"m1{cc}", name=f"m1_{cc}") for cc in range(2)]
        m2 = [main.tile([128, L], BF16, tag=f"m2{cc}", name=f"m2_{cc}") for cc in range(2)]

        def g3(ap):
            return ap.rearrange("p (h w) -> p h w", h=64)

        def hmax3(eng, dst, src):
            dv, sv = g3(dst), g3(src)
            t1 = ptmp.tile([128, L], BF16, tag="ptmp", name="ptmp")
            tv = g3(t1)
            eng.tensor_tensor(tv[:, :, 1:], sv[:, :, 1:], sv[:, :, :63], op=ALU.max)
            mp_copies.append(nc.scalar.copy(tv[:, :, 0:1], sv[:, :, 0:1]))
            eng.tensor_tensor(dv[:, :, :63], tv[:, :, :63], sv[:, :, 1:], op=ALU.max)
            mp_copies.append(nc.scalar.copy(dv[:, :, 63:64], tv[:, :, 63:64]))

        def vmax3(eng, dst, src):
            dv, sv = g3(dst), g3(src)
            t1 = ptmp.tile([128, L], BF16, tag="ptmp", name="ptmp")
            tv = g3(t1)
            eng.tensor_tensor(tv[:, 1:, :], sv[:, 1:, :], sv[:, :63, :], op=ALU.max)
            mp_copies.append(nc.scalar.copy(tv[:, 0:1, :], sv[:, 0:1, :]))
            eng.tensor_tensor(dv[:, :63, :], tv[:, :63, :], sv[:, 1:, :], op=ALU.max)
            mp_copies.append(nc.scalar.copy(dv[:, 63:64, :], tv[:, 63:64, :]))

        def hspread(eng, dst, src):   # dst[v] = max(src[v-1], src[v+1]) + edge copies
            dv, sv = g3(dst), g3(src)
            eng.tensor_tensor(dv[:, :, 1:63], sv[:, :, 0:62], sv[:, :, 2:64], op=ALU.max)
            mp_copies.append(nc.scalar.copy(dv[:, :, 0:1], sv[:, :, 1:2]))
            mp_copies.append(nc.scalar.copy(dv[:, :, 63:64], sv[:, :, 62:63]))

        def vspread(eng, dst, src):
            dv, sv = g3(dst), g3(src)
            eng.tensor_tensor(dv[:, 1:63, :], sv[:, 0:62, :], sv[:, 2:64, :], op=ALU.max)
            mp_copies.append(nc.scalar.copy(dv[:, 0:1, :], sv[:, 1:2, :]))
            mp_copies.append(nc.scalar.copy(dv[:, 63:64, :], sv[:, 62:63, :]))

        for cc in range(2):
            eng = nc.vector
            cm3 = ptmp.tile([128, L], BF16, tag="ptmp", name="ptmp")
            hmax3(eng, cm3, xT_cm[cc])
            vmax3(eng, m1[cc], cm3)
            cm5 = ptmp.tile([128, L], BF16, tag="ptmp", name="ptmp")
            hspread(eng, cm5, cm3)
            r35 = ptmp.tile([128, L], BF16, tag="ptmp", name="ptmp")
            vmax3(eng, r35, cm5)
            vspread(eng, m2[cc], r35)

        # ---- phase E: banded stencil apply, c-major out ----
        cmE = tc.tile_pool(name="psE", bufs=4, space="PSUM"); psE = cmE.__enter__()
        x1 = [main.tile([128, L], BF16, tag=f"x1{cc}", name=f"x1_{cc}") for cc in range(2)]
        x2 = [main.tile([128, L], BF16, tag=f"x2{cc}", name=f"x2_{cc}") for cc in range(2)]
        # one transpose-load per (8-chunk group, source block): the XBAR
        # un-transposes DRAM [m, k] into SBUF [k, (j, m)]; GJ = 128*384 so
        # the (j, m) row dims merge into one 1024-row input
        probe_g = {}
        for gi, jg0 in enumerate(scat_grp):
            pg = nc.vector.memset(probe_t[:, 1 + gi:2 + gi], 0.0)
            for inst in scat_edge + scat_grp[jg0]:
                add_dep_helper(pg.ins, inst.ins, reason="scatter group complete")
            probe_g[jg0] = pg
        last_mm_of_chunk = {}
        for j in range(NCHUNK):
            g = gpool.tile([128, 3, 128], BF16, tag="g", name="g")
            bs = [b for b in range(3) if 0 <= j + b - 1 < NCHUNK]
            for b in bs:
                eng = nc.sync if (b % 2 == 0) else nc.scalar
                ld = eng.dma_start_transpose(
                    g[:, b, :],
                    AP(tensor=gs_t, offset=j * GJ + 128 * b,
                       ap=[[384, 128], [1, 128]]))
                # explicit edges: tracker misses these strided DRAM ranges
                add_dep_helper(ld.ins, probe_g[(j // JG) * JG].ins,
                               reason="G load after scatter")
                # WAR: this load reuses the g buffer (gpool bufs=8) read by
                # the matmuls 8 chunks back
                if j - 8 in last_mm_of_chunk:
                    add_dep_helper(ld.ins, last_mm_of_chunk[j - 8].ins,
                                   reason="g-buffer WAR")
            for cc in range(2):
                psx = psE.tile([128, 128], F32, tag=f"psx{cc}", name="psx")
                for i, b in enumerate(bs):
                    mm = nc.tensor.matmul(
                        psx[:],
                        vT[:, j + b - 1, 128 * cc:128 * (cc + 1)],
                        g[:, b, :],
                        start=(i == 0), stop=(i == len(bs) - 1))
                last_mm_of_chunk[j] = mm
                nc.scalar.activation(x1[cc][:, 128 * j:128 * (j + 1)],
                                     psx[:], ACT.Relu)
        cmE.__exit__(None, None, None)

        if DEBUG:
            for cc in range(2):
                nc.gpsimd.dma_start(d["dbg_x1"][128 * cc:128 * (cc + 1), :], x1[cc][:])
            nc.gpsimd.dma_start(
                d["dbg_vt"][:].rearrange("p (j c) -> p j c", c=C), vT[:])

        # ---- phase G tail: x1 = relu(xr + m1); x2 = relu(x1 + m2) ----
        first_gt = True
        for n8 in range(8):
            sl = slice(512 * n8, 512 * (n8 + 1))
            for cc in range(2):
                gt = nc.vector.tensor_tensor(x1[cc][:, sl], x1[cc][:, sl],
                                             m1[cc][:, sl], op=ALU.add)
                if first_gt:
                    for cp in mp_copies:
                        add_dep_helper(gt.ins, cp.ins,
                                       reason="m-add reads maxpool edge fills")
                    first_gt = False
                nc.scalar.activation(x1[cc][:, sl], x1[cc][:, sl], ACT.Relu)
                nc.vector.tensor_tensor(x2[cc][:, sl], x1[cc][:, sl],
                                        m2[cc][:, sl], op=ALU.add)
                nc.scalar.activation(x2[cc][:, sl], x2[cc][:, sl], ACT.Relu)

        # ---- phase H: fu matmul + residual (mc-outer), BN per half ----
        cmH = tc.tile_pool(name="psH", bufs=4, space="PSUM"); psH = cmH.__enter__()
        out_all = main.tile([128, 2, L], F32, tag="out", name="out_all")
        out_cm = [out_all[:, cc, :] for cc in range(2)]
        small = es.enter_context(tc.tile_pool(name="small", bufs=1))
        bnpack = small.tile([128, 4], F32, tag="bnpack", name="bnpack")
        cins = [dram.tile([128, 2], F32, name=f"cin{m}") for m in range(2)]
        couts = [dram.tile([128, 2], F32, name=f"cout{m}") for m in range(2)]
        rhss = [x1[0], x1[1], x2[0], x2[1]]
        for mc in range(2):
            for n8 in range(8):
                sl = slice(512 * n8, 512 * (n8 + 1))
                ps = psH.tile([128, 512], F32, tag="fups", name="fups")
                for kc in range(4):
                    nc.tensor.matmul(ps[:], wfu_sb[:, kc, mc, :],
                                     rhss[kc][:, sl],
                                     start=(kc == 0), stop=(kc == 3))
                nc.scalar.activation(out_cm[mc][:, sl], ps[:], ACT.Relu,
                                     bias=bfu2[:, mc:mc + 1])
                nc.vector.tensor_tensor(out_cm[mc][:, sl], out_cm[mc][:, sl],
                                        xT_cm[mc][:, sl], op=ALU.add)
            st = small.tile([128, 8, 6], F32, tag="bnst", name="bnst")
            for n8 in range(8):
                nc.vector.bn_stats(st[:, n8, :], out_cm[mc][:, 512 * n8:512 * (n8 + 1)])
            ag = small.tile([128, 2], F32, tag="bnag", name="bnag")
            nc.vector.bn_aggr(ag[:], st[:])
            nc.vector.tensor_scalar(bnpack[:, 2 * mc:2 * mc + 1], ag[:, 0:1],
                                    float(L), None, op0=ALU.mult)
            sq = small.tile([128, 1], F32, tag="bnsq", name="bnsq")
            nc.vector.tensor_tensor(sq[:], ag[:, 0:1], ag[:, 0:1], op=ALU.mult)
            nc.vector.tensor_tensor(sq[:], sq[:], ag[:, 1:2], op=ALU.add)
            nc.vector.tensor_scalar(bnpack[:, 2 * mc + 1:2 * mc + 2], sq[:],
                                    float(L), None, op0=ALU.mult)
            nc.sync.dma_start(cins[mc][:], bnpack[:, 2 * mc:2 * mc + 2])
            nc.gpsimd.collective_compute(
                "AllReduce", ALU.add,
                replica_groups=[list(range(n_cores))],
                ins=[cins[mc].opt()], outs=[couts[mc].opt()])
        gs_sb = small.tile([128, 4], F32, tag="gsb", name="gs_sb")
        for mc in range(2):
            nc.sync.dma_start(gs_sb[:, 2 * mc:2 * mc + 2], couts[mc][:])
        NTOT = float(n_cores * L)
        scale = small.tile([128, 2], F32, tag="scale", name="scale")
        shift = small.tile([128, 2], F32, tag="shift", name="shift")
        mean = small.tile([128, 2], F32, tag="mean", name="mean")
        var = small.tile([128, 2], F32, tag="var", name="var")
        for cc in range(2):
            nc.vector.tensor_scalar(mean[:, cc:cc + 1], gs_sb[:, 2 * cc:2 * cc + 1],
                                    1.0 / NTOT, None, op0=ALU.mult)
            nc.vector.tensor_scalar(var[:, cc:cc + 1], gs_sb[:, 2 * cc + 1:2 * cc + 2],
                                    1.0 / NTOT, None, op0=ALU.mult)
        msq = small.tile([128, 2], F32, tag="msq", name="msq")
        nc.vector.tensor_tensor(msq[:], mean[:], mean[:], op=ALU.mult)
        nc.vector.tensor_tensor(var[:], var[:], msq[:], op=ALU.subtract)
        rs = small.tile([128, 2], F32, tag="rs", name="rs")
        nc.vector.tensor_scalar(var[:], var[:], float(EPS), None, op0=ALU.add)
        nc.scalar.activation(rs[:], var[:], ACT.Sqrt)
        nc.vector.reciprocal(rs[:], rs[:])
        nc.vector.tensor_tensor(scale[:], gamma2[:], rs[:], op=ALU.mult)
        nc.vector.tensor_tensor(shift[:], mean[:], scale[:], op=ALU.mult)
        nc.vector.tensor_tensor(shift[:], beta2[:], shift[:], op=ALU.subtract)

        # normalize in place, DMA out c-major (host un-transposes)
        for n8 in range(8):
            sl = slice(512 * n8, 512 * (n8 + 1))
            for cc in range(2):
                nc.vector.tensor_scalar(out_cm[cc][:, sl], out_cm[cc][:, sl],
                                        scale[:, cc:cc + 1], shift[:, cc:cc + 1],
                                        op0=ALU.mult, op1=ALU.add)
                eng = nc.sync if (n8 % 2 == 0) else nc.scalar
                eng.dma_start(d["y"][128 * cc:128 * (cc + 1), sl],
                              out_cm[cc][:, sl])
        cmH.__exit__(None, None, None)


_CACHE = {}


def _get_program(n_cores=N_CORES):
    key = n_cores
    if key not in _CACHE:
        nc = bacc.Bacc("TRN2", target_bir_lowering=False, debug=False,
                       num_devices=n_cores)
        build(nc, n_cores)
        nc.compile()
        _CACHE[key] = nc
    return _CACHE[key]


_CONSTS = None


def make_in_map(inputs, b):
    global _CONSTS
    if _CONSTS is None:
        _CONSTS = host_consts()
    import ml_dtypes
    # pre-permute to the transposed-grid token order l' = w*64 + h
    xbf = np.ascontiguousarray(
        np.asarray(inputs["x"][b]).transpose(1, 0, 2).reshape(L, C)
    ).astype(ml_dtypes.bfloat16)
    return {
        "xbf": xbf,
        "wv": np.ascontiguousarray(inputs["Wv"], np.float32),
        "bv": np.ascontiguousarray(np.asarray(inputs["bv"]).reshape(1, C), np.float32),
        "wa": np.ascontiguousarray(inputs["Wa"], np.float32),
        "ba": np.ascontiguousarray(np.asarray(inputs["ba"]).reshape(81, 1), np.float32),
        "wfu": np.ascontiguousarray(inputs["Wfu"], np.float32),
        "bfu2": np.ascontiguousarray(
            np.asarray(inputs["bfu"]).reshape(2, 128).T, np.float32),
        "gamma2": np.ascontiguousarray(
            np.asarray(inputs["gamma"]).reshape(2, 128).T, np.float32),
        "beta2": np.ascontiguousarray(
            np.asarray(inputs["beta"]).reshape(2, 128).T, np.float32),
        **_CONSTS,
    }


def postprocess(yarr):
    """[256, L] c-major, l' = w*64+h  ->  [H, W, C] in the reference frame."""
    return np.asarray(yarr, np.float32).reshape(C, L).T.reshape(H, W, C)


def kernel(**inputs):
    nc = _get_program()
    in_maps = [make_in_map(inputs, b) for b in range(B)]
    res = run_bass_kernel_spmd(nc, in_maps, list(range(N_CORES)))
    out = np.stack([postprocess(res.results[b]["y"]) for b in range(B)])
    return out.astype(np.float32)
